# revision 1
# baseline (speedup 1.0000x reference)
"""Trainium2 Bass kernel for the reaction-wheel encoder elementwise problem.

Reference semantics (per element, f32 unless noted):
    temp   = wheel_speeds * K + remaining_clicks        (K = DT * CPR, f32)
    clicks = trunc(temp)
    nominal_out = clicks * (1/K)        [reference: clicks / K]
    nominal_rem = temp - clicks
    state == 0 (nominal): out = nominal_out, rem = nominal_rem
    state == 1 (off):     out = 0,           rem = 0
    state == 2 (stuck):   out = converted,   rem = remaining_clicks

Sharding: contiguous 1/8 slices across the 8 NeuronCores (pure data
parallel).  Host-side the four input streams are interleaved per [128, FD]
tile into one packed DRAM tensor per core ([nt, 128, 4, FD], slice order
ws, rc, cv, state; the int32 state rides along as an f32 bit pattern).  Each
loop iteration issues the input as two 2 MiB DMAs — (ws, rc) first so the
DVE can start before (cv, state) lands.  The outputs are packed [rem | out]
in one [nt, 128, 2, FD] tensor (slice order matches the (rc, cv) input
adjacency so one wide copy_predicated handles both stuck-overrides), stored
with a single 2 MiB DMA.

The kernel is raw bass (not Tile): this toolchain's walrus accepts at most
one attached sync-wait per instruction, so cross-engine ordering uses
standalone engine-queue wait_ge instructions with hand-assigned semaphores.
Every DMA gets its own per-buffer-slot semaphore (a DMA's 16 increments come
from the 16 SDMA engines independently, so concurrent DMAs may not share
one).

trunc(x): every f32->i32 convert path on this hardware rounds to nearest
even, so trunc is built from fp arithmetic (all exact, |x| < 2^22):
    rn  = (x + 1.5*2^23) - 1.5*2^23          # RNE-to-integer
    d   = x - rn                             # in [-0.5, 0.5]
    rem = d + (d*sign(x) < 0) * sign(x)      # toward-zero correction
    clicks = x - rem
The rn step is one fused two-op tensor_scalar (2x perf mode), sign(x) is a
scale-and-clamp tensor_scalar pair (2x), the correction is one custom DVE op
(7 ALU slices), clicks*invK a second custom op.

Engine split per tile:
    DVE: temp = ws*K + rc (AFFINE_THEN_ADD); sign; rem (custom);
         out = (temp-rem)*invK (custom); [rem|out] *= [m0|m0];
         copy_predicated([rem|out], [m2|m2], [rc|cv])
    ACT: m0 = Relu(1 - s) f32 twice; m2 = Relu(s - 1) i32 twice
"""

import os
import sys

import numpy as np

for _p in ("/opt/trn_rl_repo", os.path.expanduser("~/.axon_site/_ro/trn_rl_repo")):
    if os.path.isdir(_p) and _p not in sys.path:
        sys.path.insert(0, _p)

import concourse.bass as bass
import concourse.mybir as mybir
import concourse.dve_ops as dve_ops
from concourse.dve_spec import C0 as _C0
from concourse.dve_spec import Spec, Src0, Src1, Zero, lower, _has_src1
from concourse.dve_uop import DveOpSpec
from concourse.bass_utils import run_bass_kernel_spmd

N_TOTAL = 16_777_216
N_CORES = 8
PER_CORE = N_TOTAL // N_CORES  # 2,097,152
P = 128
FD = 2048  # free-dim columns per tile
NT = PER_CORE // (P * FD)  # 8 tiles/core
BUFS = 2       # compute/output tile slots
BUFS_IN = 3    # input tile slots (deeper so input DMAs are hidden)

F32 = mybir.dt.float32
I32 = mybir.dt.int32
I8 = mybir.dt.int8
U8 = mybir.dt.uint8
ALU = mybir.AluOpType
ACT = mybir.ActivationFunctionType

# Match the reference's f32 scalar constant exactly: jax multiplies the f32
# array by the python double DT*CPR, which downcasts to f32 first.
K32 = np.float32(0.1 * (2048.0 / (2.0 * np.pi)))
INVK32 = np.float32(1.0) / K32
MAGIC = float(np.float32(1.5 * 2.0**23))  # RNE-to-int shifter, |x| < 2^22


def _register_custom_op(name, spec):
    """Append a custom DVE op to the module-level registry, self-pinning its
    lowered-uop sha (we author for this process, not a frozen fleet)."""
    for op in dve_ops.OPS:
        if op.name == name:
            return op
    row = dve_ops._CUSTOM_DVE_ROW_BASE + len(dve_ops.OPS)
    assert row < 0x20
    dve_ops._SUB_OPCODE_FOR_NAME[name] = row
    shas = {}
    for ver in ("v3", "v4"):
        try:
            tmp = DveOpSpec(
                name=name, opcode=row, uops=lower(spec, ver=ver),
                rd1_en=_has_src1(spec),
            )
            shas[ver] = tmp.sha(ver)
        except Exception:
            pass
    op = dve_ops.DveOp(name, spec, subdim=False, uops_sha=shas)
    dve_ops.OPS.append(op)
    dve_ops.CUSTOM_DVE_SPECS[name] = spec
    return op


def _rem_trunc_ref(in0, in1, s0, s1, imm2):
    x = in0.astype(np.float32)
    sgn = in1.astype(np.float32)
    rn = ((x + np.float32(s0)) - np.float32(s0)).astype(np.float32)
    d = (x - rn).astype(np.float32)
    away = ((d * sgn).astype(np.float32) < 0).astype(np.float32)
    return (d + away * sgn).astype(np.float32)


# Src0 = temp, Src1 = sign(temp) (+-1; magnitude only matters when |temp|>0.5),
# C0 = 1.5*2^23.  rem = d + (d*s < 0)*s with d = temp - ((temp+C0)-C0).
# Strict 7-op dependency chain -> schedules into 7 of the 8 ALU slices.
_d = (Src0 - ((Src0 + _C0) - _C0))
REM_TRUNC = _register_custom_op(
    "REM_TRUNC_ANT",
    Spec(
        body=_d + ((_d * Src1) < Zero) * Src1,
        reference=_rem_trunc_ref,
    ),
)

# out = (x - rem) * invK   [Src0=x, Src1=rem, C0=invK]
CLICKS_SCALE = _register_custom_op(
    "CLICKS_SCALE_ANT",
    Spec(
        body=(Src0 - Src1) * _C0,
        reference=lambda in0, in1, s0, s1, imm2: (
            (in0.astype(np.float32) - in1.astype(np.float32)) * np.float32(s0)
        ).astype(np.float32),
    ),
)


def build_nc(nt: int = NT, fd: int = FD) -> bass.Bass:
    nc = bass.Bass()
    # byte-packed input row: ws(4*fd) | rc(4*fd) | cv(4*fd) | state-int8(fd)
    in_d = nc.dram_tensor("packed_in", [nt, P, 13 * fd], U8, kind="ExternalInput")
    out_d = nc.dram_tensor("packed_out", [nt, P, 2, fd], F32, kind="ExternalOutput")
    in_v, out_v = in_d[:], out_d[:]

    with nc.sbuf_tensor("t_in", [P, BUFS_IN, 13 * fd], U8) as t_in, \
         nc.sbuf_tensor("t_tmp", [P, 1, fd], F32) as t_tmp, \
         nc.sbuf_tensor("t_x", [P, BUFS, fd], F32) as t_x, \
         nc.sbuf_tensor("t_sgn", [P, BUFS, fd], F32) as t_sgn, \
         nc.sbuf_tensor("t_or", [P, BUFS, 2, fd], F32) as t_or, \
         nc.sbuf_tensor("t_m0", [P, BUFS, fd], F32) as t_m0, \
         nc.sbuf_tensor("t_m2", [P, BUFS, 2, fd], I8) as t_m2, \
         nc.sbuf_tensor("t_neg1", [P, 1], F32) as t_neg1:
        # one sem per (slot, half) input DMA and per slot output DMA
        s_in_a = [nc.semaphore(name=f"s_ina{b}").__enter__() for b in range(BUFS_IN)]
        s_in_b = [nc.semaphore(name=f"s_inb{b}").__enter__() for b in range(BUFS_IN)]
        s_in_c = [nc.semaphore(name=f"s_inc{b}").__enter__() for b in range(BUFS_IN)]
        s_out = [nc.semaphore(name=f"s_out{b}").__enter__() for b in range(BUFS)]
        # s_dve tick order: x0=1, x1=2, pred0=3, x2=4, pred1=5, ... so
        # x(i) done <=> s_dve >= max(1, 2i) and pred(j) done <=> s_dve >=
        # 2j+3.  s_act ticks per iter: m0(i)=4i+1, m2=4i+2..3, sign=4i+4.
        s_dve = nc.semaphore(name="s_dve").__enter__()
        s_act = nc.semaphore(name="s_act").__enter__()
        s_ini = nc.semaphore(name="s_ini").__enter__()  # const init

        # Chunk schedule: the first and last tiles are split into column
        # chunks so the pipeline fills and drains at fine granularity
        # (virtual iteration v = (tile, col offset, width)).
        # Split only the FIRST tile into halves: the pipeline-fill chain
        # (2 MiB DMA -> affine -> masks -> sign) starts on 1 MiB instead.
        # (Measured on HW: chunking both ends into quarters cost more in
        # per-op overhead + small-DMA inefficiency than it saved.)
        if nt >= 2 and fd % 2 == 0:
            h = fd // 2
            sched = [(0, 0, h), (0, h, h)] + [(t, 0, fd) for t in range(1, nt)]
        else:
            sched = [(t, 0, fd) for t in range(nt)]
        nv = len(sched)
        # per-input-slot DMA-use counters -> wait targets per v
        ka = [0] * nv
        kb = [0] * nv
        kc = [0] * nv
        cnt = {"a": [0] * BUFS_IN, "b": [0] * BUFS_IN, "c": [0] * BUFS_IN}

        def dma_in(v):
            t, c, w = sched[v]
            b = v % BUFS_IN
            if w == fd:
                # ws+rc in one contiguous 8*fd-byte DMA
                nc.sync.dma_start(
                    t_in.ap()[:, b, 0 : 8 * fd], in_v[t, :, 0 : 8 * fd]
                ).then_inc(s_in_a[b], 16)
                cnt["a"][b] += 1
                ka[v] = 16 * cnt["a"][b]
                # cv + int8 state contiguous
                nc.sync.dma_start(
                    t_in.ap()[:, b, 8 * fd : 13 * fd],
                    in_v[t, :, 8 * fd : 13 * fd],
                ).then_inc(s_in_b[b], 16)
                cnt["b"][b] += 1
                kb[v] = 16 * cnt["b"][b]
                kc[v] = 0
            else:
                # ws+rc column chunk: two 4w-byte ranges at stride 4*fd
                src = in_v[t, :, 4 * c : 4 * c + 8 * fd].rearrange(
                    "p (a z) -> p a z", a=2
                )[:, :, 0 : 4 * w]
                dst = t_in.ap()[:, b, 4 * c : 4 * c + 8 * fd].rearrange(
                    "p (a z) -> p a z", a=2
                )[:, :, 0 : 4 * w]
                nc.sync.dma_start(dst, src).then_inc(s_in_a[b], 16)
                cnt["a"][b] += 1
                ka[v] = 16 * cnt["a"][b]
                nc.sync.dma_start(
                    t_in.ap()[:, b, 8 * fd + 4 * c : 8 * fd + 4 * c + 4 * w],
                    in_v[t, :, 8 * fd + 4 * c : 8 * fd + 4 * c + 4 * w],
                ).then_inc(s_in_b[b], 16)
                cnt["b"][b] += 1
                kb[v] = 16 * cnt["b"][b]
                nc.sync.dma_start(
                    t_in.ap()[:, b, 12 * fd + c : 12 * fd + c + w],
                    in_v[t, :, 12 * fd + c : 12 * fd + c + w],
                ).then_inc(s_in_c[b], 16)
                cnt["c"][b] += 1
                kc[v] = 16 * cnt["c"][b]

        # ---- SP queue: all DMAs -------------------------------------------
        for v in range(min(BUFS_IN, nv)):
            dma_in(v)
        for v in range(nv):
            t, c, w = sched[v]
            s = v % BUFS
            nc.sync.wait_ge(s_dve, 2 * v + 3)  # pred(v) done
            if w == fd:
                dst = out_v[t]
            else:
                dst = out_v[t][:, :, c : c + w]
            nc.sync.dma_start(
                dst, t_or.ap()[:, s, :, 0:w]
            ).then_inc(s_out[s], 16)
            if v + BUFS_IN < nv:
                # pred(v) done implies DVE (and transitively ACT) is finished
                # with input slot v % BUFS_IN
                dma_in(v + BUFS_IN)

        # ---- DVE queue (software-pipelined: pred runs one step behind) ----
        nc.vector.memset(t_neg1.ap(), -1.0)
        nc.vector.drain()
        nc.vector.nop().then_inc(s_ini, 1)
        for v in range(nv + 1):
            s = v % BUFS
            si = v % BUFS_IN
            if v < nv:
                t, c, w = sched[v]
                # temp(v) = ws*K + rc
                nc.vector.wait_ge(s_in_a[si], ka[v])
                nc.vector.affine_then_add(
                    out=t_tmp.ap()[:, 0, 0:w],
                    in0=t_in.ap()[:, si, 4 * c : 4 * c + 4 * w].bitcast(F32),
                    in1=t_in.ap()[
                        :, si, 4 * fd + 4 * c : 4 * fd + 4 * c + 4 * w
                    ].bitcast(F32),
                    scale=float(K32), bias=0.0,
                )
                nc.vector.drain()
                # x(v) = temp * m0: masked lanes collapse to +-0 through the
                # whole trunc pipeline, so no output-side m0 multiply needed
                nc.vector.wait_ge(s_act, 4 * v + 1)
                nc.vector.tensor_tensor(
                    out=t_x.ap()[:, s, 0:w], in0=t_tmp.ap()[:, 0, 0:w],
                    in1=t_m0.ap()[:, s, 0:w], op=ALU.mult,
                )
                nc.vector.drain()
                nc.vector.nop().then_inc(s_dve, 1)  # tick max(1, 2v)
            if v >= 1:
                # pred(v-1): stuck overrides (fills the sign(v) wait gap)
                j = v - 1
                tj, cj, wj = sched[j]
                sj = j % BUFS
                sij = j % BUFS_IN
                nc.vector.wait_ge(s_in_b[sij], kb[j])
                if wj == fd:
                    nc.vector.copy_predicated(
                        out=t_or.ap()[:, sj], mask=t_m2.ap()[:, sj],
                        data=t_in.ap()[:, sij, 4 * fd : 12 * fd].bitcast(F32),
                    )
                else:
                    for h in range(2):
                        off = (4 + 4 * h) * fd + 4 * cj
                        nc.vector.copy_predicated(
                            out=t_or.ap()[:, sj, h, 0:wj],
                            mask=t_m2.ap()[:, sj, h, 0:wj],
                            data=t_in.ap()[:, sij, off : off + 4 * wj].bitcast(
                                F32
                            ),
                        )
                nc.vector.drain()
                nc.vector.nop().then_inc(s_dve, 1)  # pred(j) done: tick 2j+3
                if v == nv:
                    # no x(nv): keep the tick arithmetic uniform
                    nc.vector.nop().then_inc(s_dve, 1)
            if v < nv:
                o_rem = t_or.ap()[:, s, 0, 0:w]
                o_out = t_or.ap()[:, s, 1, 0:w]
                if v >= BUFS:
                    nc.vector.wait_ge(s_out[s], 16 * (v // BUFS))
                # rem(v) = x - trunc(x)  (RNE magic + toward-zero fix, fused)
                nc.vector.wait_ge(s_act, 4 * v + 4)  # sign(v) ready
                nc.vector._custom_dve(
                    REM_TRUNC, out=o_rem,
                    in0=t_x.ap()[:, s, 0:w], in1=t_sgn.ap()[:, s, 0:w],
                    s0=MAGIC,
                )
                nc.vector.drain()
                # out(v) = (x - rem) * invK = trunc(x) * invK
                nc.vector._custom_dve(
                    CLICKS_SCALE, out=o_out,
                    in0=t_x.ap()[:, s, 0:w], in1=o_rem, s0=float(INVK32),
                )
                nc.vector.drain()

        # ---- ACT queue: masks + sign --------------------------------------
        nc.scalar.wait_ge(s_ini, 1)
        for v in range(nv):
            t, c, w = sched[v]
            s = v % BUFS
            si = v % BUFS_IN
            st = t_in.ap()[:, si, 12 * fd + c : 12 * fd + c + w].bitcast(I8)
            if w == fd:
                nc.scalar.wait_ge(s_in_b[si], kb[v])
            else:
                nc.scalar.wait_ge(s_in_c[si], kc[v])
            if v >= BUFS:
                # mask slots re-used; pred(v-2) was their last reader
                nc.scalar.wait_ge(s_dve, 2 * (v - 2) + 3)
            nc.scalar.activation(
                t_m0.ap()[:, s, 0:w], st, ACT.Relu, bias=1.0, scale=-1.0
            )
            nc.scalar.drain()
            nc.scalar.nop().then_inc(s_act, 1)
            for h in range(2):
                nc.scalar.activation(
                    t_m2.ap()[:, s, h, 0:w], st, ACT.Relu,
                    bias=t_neg1.ap(), scale=1.0,
                )
                nc.scalar.drain()
                nc.scalar.nop().then_inc(s_act, 1)
            # sign(x(v)) via the Sign table; +-0 inputs give +-0 which the
            # custom op treats as "no correction" (correct for masked lanes)
            nc.scalar.wait_ge(s_dve, max(1, 2 * v))
            nc.scalar.activation(
                t_sgn.ap()[:, s, 0:w], t_x.ap()[:, s, 0:w], ACT.Sign,
                bias=0.0, scale=1.0,
            )
            nc.scalar.drain()
            nc.scalar.nop().then_inc(s_act, 1)

    # Raw bass skips Bacc's extended-inst lowering; without it the custom
    # DVE instructions reach walrus with empty .instr ("ISA wrong length").
    mybir.codegen_inst_isa_subclasses(nc)
    nc.finalize()
    return nc


_NC_CACHE: bass.Bass | None = None


def _get_nc() -> bass.Bass:
    global _NC_CACHE
    if _NC_CACHE is None:
        _NC_CACHE = build_nc()
    return _NC_CACHE


def make_in_maps(wheel_speeds, remaining_clicks, converted, rw_signal_state):
    """Shard + byte-pack the full inputs into per-core packed_in arrays.

    Per (tile, partition) row: ws | rc | cv as f32 bytes, then the signal
    state squeezed to int8 (it only holds 0/1/2) - saves 0.75 MiB of HBM
    read traffic per tile."""
    u8 = np.uint8
    ws = np.asarray(wheel_speeds, dtype=np.float32).reshape(N_CORES, NT, P, FD)
    rc = np.asarray(remaining_clicks, dtype=np.float32).reshape(N_CORES, NT, P, FD)
    cv = np.asarray(converted, dtype=np.float32).reshape(N_CORES, NT, P, FD)
    st8 = np.asarray(rw_signal_state, dtype=np.int32).astype(np.int8)
    packed = np.concatenate(
        [
            ws.view(u8).reshape(N_CORES, NT, P, 4 * FD),
            rc.view(u8).reshape(N_CORES, NT, P, 4 * FD),
            cv.view(u8).reshape(N_CORES, NT, P, 4 * FD),
            st8.view(u8).reshape(N_CORES, NT, P, FD),
        ],
        axis=3,
    )  # [cores, nt, P, 13*FD]
    return [{"packed_in": np.ascontiguousarray(packed[c])} for c in range(N_CORES)]


def unpack_results(results):
    po = np.stack([results[c]["packed_out"] for c in range(N_CORES)], axis=0)
    po = po.reshape(N_CORES, NT, P, 2, FD)
    rem = np.ascontiguousarray(po[:, :, :, 0, :]).reshape(N_TOTAL)
    out = np.ascontiguousarray(po[:, :, :, 1, :]).reshape(N_TOTAL)
    return out, rem


def kernel(wheel_speeds, remaining_clicks, converted, rw_signal_state):
    nc = _get_nc()
    in_maps = make_in_maps(wheel_speeds, remaining_clicks, converted, rw_signal_state)
    res = run_bass_kernel_spmd(nc, in_maps, core_ids=list(range(N_CORES)))
    return unpack_results(res.results)

